# revision 28
# baseline (speedup 1.0000x reference)
"""Trainium2 Bass kernel for nn_ContrastiveLoss (B=4096, F=256, T=0.1).

Circulant-symmetric strategy (8 NeuronCores, identical SPMD program):
  - the 64 row-tiles (128 rows each) of the combined normalized matrix are
    assigned 8-per-core: core k owns absolute tiles 8k..8k+7. Every
    unordered tile pair {i, j} is computed exactly once, oriented by the
    circulant offset d = (j - i) mod 64: row i computes tiles d = 0..32.
    d = 0 (diag tile) and d = 32 (paired both ways) contribute row-sums
    only; d = 1..31 contribute row-sums AND column-sums (the transposed
    half), recovered with ones-vector matmuls chained in PSUM.
  - core k loads only the 40 column-tiles (8k + j) mod 64, j = 0..39, in
    permuted order (the host pre-arranges `cols_raw`), normalizes them to
    bf16 on DVE (squares + Newton-rsqrt + scale; TensorScalarPtr is not a
    legal Pool opcode, so GpSimd cannot help), and xbar-transposes into cT
    [256, 5120]. Loads and transposes interleave on the SP HWDGE ring in
    4-tile groups so the first exp starts as early as possible.
  - per own row q: 3-4 PSUM strips of <=1536 cols, exp'd by ScalarE with
    fused row-sum accumulation; exp tiles (bf16) are column-summed by
    M=1 ones-matmuls into 2 rotating PSUM banks per global 512-chunk.
  - host finishes in float64: E_i = rowsum_i + colsum_i - exp(10*d_i),
    lse_i = log(E_i), gf = sum_k gb_k, neg = 10*(|gf|^2 - sum d) -
    (2B-1)*sum lse, loss = -mean(pos)/T + neg/(4B^2).
"""

import sys

sys.path.insert(0, "/opt/trn_rl_repo")

from contextlib import ExitStack  # noqa: E402

import numpy as np  # noqa: E402

import concourse.bass as bass  # noqa: E402
import concourse.mybir as mybir  # noqa: E402
import concourse.tile as tile  # noqa: E402
from concourse import bacc  # noqa: E402
from concourse.bass_utils import run_bass_kernel_spmd  # noqa: E402

B = 4096
F = 256
TWO_B = 2 * B
N_CORES = 8
INV_T = 10.0
EPS2 = 1e-14

F32 = mybir.dt.float32
BF16 = mybir.dt.bfloat16
U32 = mybir.dt.uint32
OP = mybir.AluOpType

NLOAD = 40  # column-tiles loaded per core (permuted order)
NROWS = 8  # own row-tiles per core
D_MAX = 32  # largest circulant offset computed (inclusive)
ROW_W = 128 * (D_MAX + 1)  # 4224 cols computed per own row
CS_LO_REL = 128  # cs range per row, relative to row start
CS_HI_REL = 128 * D_MAX  # 4096: d=32 tile excluded from colsums
SW = 1536  # PSUM strip width (3 banks)
N_STRIP = 3  # strips per row: 1536+1536+1152
GLOB_W = 128 * NLOAD  # 5120 permuted columns
CS_GLO = 128  # global permuted colsum range [128, 4992)
CS_GHI = CS_HI_REL + 128 * (NROWS - 1)  # 4992
CS_W = CS_GHI - CS_GLO  # 4864 shipped colsum values

# stats tile layout (columns); pos dots and the global row-sum vector gf are
# recomputed on the host in float64 (cheap, and closer to the reference)
S_E = 0  # 0:8   exp row-sums per own row-tile
S_D = 8  # 8:16  d_i = ||c_i||^2 for own rows
S_W = 16


N_STRIP_MAX = 4  # row 0 uses 4 strips (early-start split), others 3


def _row_bounds(q):
    # strip boundaries at GLOBAL multiples of 1536: every row's first piece
    # then ends at col 1536, so all 8 first pieces (~8.7us of exp work)
    # unlock with just the first three 4-tile transpose groups — the ACT
    # stream starts early and never starves waiting for later transposes
    r0, r1 = 128 * q, 128 * q + ROW_W
    return [r0] + [b for b in (SW, 2 * SW, 3 * SW) if r0 < b < r1] + [r1]


def _pieces():
    """Static (start, end, q, t) list of per-row PSUM strips, sorted by
    global permuted start column."""
    ps = []
    for q in range(NROWS):
        b = _row_bounds(q)
        for t in range(len(b) - 1):
            ps.append((b[t], b[t + 1], q, t))
    ps.sort()
    return ps


def _chains(pieces):
    """Per global 512-chunk c: list of (piece_idx, q, lo, hi) colsum
    segments, plus the piece index after which the chain can be emitted."""
    n_chunk = (CS_GHI + 511) // 512  # 10
    chains = []
    for c in range(n_chunk):
        glo, ghi = 512 * c, 512 * (c + 1)
        segs = []
        last_pi = -1
        for pi, (ps, pe, q, t) in enumerate(pieces):
            lo = max(glo, ps, 128 * q + CS_LO_REL)
            hi = min(ghi, pe, 128 * q + CS_HI_REL)
            if lo < hi:
                segs.append((pi, q, lo, hi))
                last_pi = max(last_pi, pi)
        assert segs
        lo_u = min(s[2] for s in segs)
        hi_u = max(s[3] for s in segs)
        chains.append((c, lo_u, hi_u, segs, last_pi))
    return chains


def _build_kernel(loop_n=None):
    nc = bacc.Bacc("TRN2", target_bir_lowering=False, debug=False, num_devices=N_CORES)

    cols_raw = nc.dram_tensor("cols_raw", [GLOB_W, F], F32, kind="ExternalInput")
    out = nc.dram_tensor("out", [128, S_W], F32, kind="ExternalOutput")
    colsum = nc.dram_tensor("colsum", [1, CS_W], F32, kind="ExternalOutput")

    with tile.TileContext(nc) as tc, ExitStack() as octx:
        if loop_n is not None:
            octx.enter_context(tc.For_i(0, loop_n, 1))
        _emit_body(nc, tc, cols_raw, out, colsum)

    nc.compile()
    return nc


def _emit_body(nc, tc, cols_raw, out, colsum):
    pieces = _pieces()
    chains = _chains(pieces)
    # chain -> emit after this piece index (one piece of lag so the PE is
    # never parked behind the producing ACT in its FIFO)
    emit_after = {}
    for c, lo_u, hi_u, segs, last_pi in chains:
        emit_after.setdefault(min(last_pi + 1, len(pieces) - 1), []).append(
            (c, lo_u, hi_u, segs)
        )

    piece_et = {}
    with ExitStack() as ctx:
        singles = ctx.enter_context(tc.tile_pool(name="singles", bufs=1))
        scr = ctx.enter_context(tc.tile_pool(name="scr", bufs=2))
        etp = ctx.enter_context(tc.tile_pool(name="etp", bufs=16))
        mm = ctx.enter_context(tc.tile_pool(name="mm", bufs=2, space="PSUM"))
        csp = ctx.enter_context(tc.tile_pool(name="csp", bufs=2, space="PSUM"))

        stats = singles.tile([128, S_W], F32)
        raw = singles.tile([128, NLOAD, F], F32)
        scaled = singles.tile([128, 2, NLOAD, 128], BF16)  # chunk-major
        cT = [singles.tile([128, GLOB_W], BF16, name=f"cT{c}") for c in range(2)]
        ss = singles.tile([128, NLOAD], F32)
        y = singles.tile([128, NLOAD], F32)
        e_parts = singles.tile([128, NROWS * N_STRIP_MAX], F32)
        cs_sb = singles.tile([1, CS_W], F32)
        ones = singles.tile([128, 1], BF16)
        magicf = singles.tile([128, 8], F32)
        warm = singles.tile([128, 1], F32)
        nc.vector.memset(magicf[:], float(0x5F3759DF))
        nc.vector.memset(ones[:], 1.0)
        nc.vector.memset(warm[:], 0.0)
        # pull the exp ACT-table DMA to t=0, ahead of the bulk loads
        nc.scalar.activation(
            warm[:], warm[:], mybir.ActivationFunctionType.Exp, bias=0.0, scale=1.0
        )

        # ---- loads + transposes share the SP HWDGE ring, interleaved so the
        # first two transpose groups run as soon as their norms are done,
        # before the remaining bulk loads occupy the DMA engines ------------
        src = cols_raw.ap().rearrange("(t p) f -> p t f", p=128)

        def load(g):
            nc.sync.dma_start(raw[:, 4 * g : 4 * g + 4, :], src[:, 4 * g : 4 * g + 4, :])

        def norm_group(g):
            """normalize tiles 4g..4g+4, Newton-rsqrt + scale on DVE.
            Sum-of-squares: the first four groups ride the otherwise-idle
            ScalarE front (Square is in the exp_and_others table set, so no
            table switch); later groups use DVE with a bf16 throwaway out
            (half the write width; ss keeps fp32 accum precision)."""
            for j in range(4 * g, 4 * g + 4):
                if g < 4:
                    sqb = scr.tile([128, F], BF16, tag="sqb")
                    nc.scalar.activation(
                        sqb[:], raw[:, j, :], mybir.ActivationFunctionType.Square,
                        bias=0.0, scale=1.0,
                        accum_out=ss[:, j : j + 1],
                    )
                else:
                    sqb = scr.tile([128, F], BF16, tag="sqb")
                    nc.vector.scalar_tensor_tensor(
                        out=sqb[:], in0=raw[:, j, :], scalar=0.0, in1=raw[:, j, :],
                        op0=OP.bypass, op1=OP.mult,
                        accum_out=ss[:, j : j + 1],
                    )
            sl = slice(4 * g, 4 * g + 4)
            nc.vector.tensor_scalar_max(ss[:, sl], ss[:, sl], EPS2)
            bits_f = scr.tile([128, 4], F32, tag="hb")
            nc.vector.tensor_copy(bits_f[:], ss[:, sl].bitcast(U32))
            seed_f = scr.tile([128, 4], F32, tag="sf")
            nc.vector.scalar_tensor_tensor(
                out=seed_f[:], in0=bits_f[:], scalar=-0.5, in1=magicf[:, :4],
                op0=OP.mult, op1=OP.add,
            )
            nc.vector.tensor_copy(y[:, sl].bitcast(U32), seed_f[:])
            for _ in range(2):
                t1 = scr.tile([128, 4], F32, tag="nr")
                nc.vector.tensor_tensor(t1[:], y[:, sl], y[:, sl], OP.mult)
                t2 = scr.tile([128, 4], F32, tag="nr")
                nc.vector.scalar_tensor_tensor(
                    out=t2[:], in0=t1[:], scalar=-0.5, in1=ss[:, sl],
                    op0=OP.mult, op1=OP.mult,
                )
                t3 = scr.tile([128, 4], F32, tag="nr")
                nc.vector.tensor_scalar_add(t3[:], t2[:], 1.5)
                nc.vector.tensor_tensor(y[:, sl], y[:, sl], t3[:], OP.mult)
            for j in range(4 * g, 4 * g + 4):
                nc.vector.tensor_scalar_mul(
                    scaled[:, :, j, :],
                    raw[:, j, :].rearrange("p (c f) -> p c f", c=2),
                    y[:, j : j + 1],
                )

        def transpose_group(g):
            """xbar-transpose tiles 4g..4g+4 into cT columns (SP ring)."""
            for c in range(2):
                nc.sync.dma_start_transpose(
                    out=cT[c][:, 512 * g : 512 * g + 512].rearrange(
                        "p (t m) -> p t m", m=128
                    ),
                    in_=scaled[:, c, 4 * g : 4 * g + 4, :],
                )

        for g in range(3):
            load(g)
        for g in range(3):
            norm_group(g)
            transpose_group(g)
        for g in range(3, 10):
            load(g)
            norm_group(g)
            transpose_group(g)

        # ---- side stats (off critical path) --------------------------------
        for q in range(NROWS):
            sqd = scr.tile([128, 2, 128], F32, tag="sqd")
            nc.vector.scalar_tensor_tensor(
                out=sqd[:], in0=scaled[:, :, q, :], scalar=0.0,
                in1=scaled[:, :, q, :], op0=OP.bypass, op1=OP.mult,
                accum_out=stats[:, S_D + q : S_D + q + 1],
            )

        # ---- main loop: pieces in global column order ----------------------
        for pi, (ps, pe, q, t) in enumerate(pieces):
            w = pe - ps
            pt = mm.tile([128, SW], F32, tag="mmt")
            for c in range(2):
                lhsT = cT[c][:, 128 * q : 128 * q + 128]
                for h0 in range(0, w, 512):
                    h1 = min(w, h0 + 512)
                    nc.tensor.matmul(
                        pt[:, h0:h1],
                        lhsT,
                        cT[c][:, ps + h0 : ps + h1],
                        start=(c == 0),
                        stop=(c == 1),
                    )
            et = etp.tile([128, SW], BF16, tag="et")
            idx = q * N_STRIP_MAX + t
            nc.scalar.activation(
                et[:, :w], pt[:, :w], mybir.ActivationFunctionType.Exp,
                bias=0.0, scale=INV_T,
                accum_out=e_parts[:, idx : idx + 1],
            )
            piece_et[pi] = (et, ps)

            for c, lo_u, hi_u, segs in emit_after.get(pi, []):
                cst = csp.tile([128, 512], F32, tag="cs")
                for si, (spi, sq_, lo, hi) in enumerate(segs):
                    set_, sps = piece_et[spi]
                    nc.tensor.matmul(
                        cst[0:1, lo - 512 * c : hi - 512 * c],
                        ones[:, 0:1],
                        set_[:, lo - sps : hi - sps],
                        start=(si == 0),
                        stop=(si == len(segs) - 1),
                    )
                nc.vector.tensor_copy(
                    cs_sb[0:1, lo_u - CS_GLO : hi_u - CS_GLO],
                    cst[0:1, lo_u - 512 * c : hi_u - 512 * c],
                )
                if c == 7:
                    # bulk of the colsum output can ship while the last two
                    # chains are still accumulating
                    nc.sync.dma_start(
                        colsum.ap()[0:1, 0 : 8 * 512 - CS_GLO],
                        cs_sb[0:1, 0 : 8 * 512 - CS_GLO],
                    )

        # ---- remaining reductions ------------------------------------------
        for q in range(NROWS):
            nc.vector.tensor_reduce(
                stats[:, S_E + q : S_E + q + 1],
                e_parts[:, q * N_STRIP_MAX : q * N_STRIP_MAX + len(_row_bounds(q)) - 1],
                mybir.AxisListType.X, OP.add,
            )

        nc.sync.dma_start(out.ap(), stats[:])
        nc.sync.dma_start(
            colsum.ap()[0:1, 8 * 512 - CS_GLO : CS_W],
            cs_sb[0:1, 8 * 512 - CS_GLO : CS_W],
        )


_NC_CACHE = None


def _get_nc():
    global _NC_CACHE
    if _NC_CACHE is None:
        _NC_CACHE = _build_kernel()
    return _NC_CACHE


def make_in_maps(first, second):
    f = np.ascontiguousarray(first, dtype=np.float32)
    s = np.ascontiguousarray(second, dtype=np.float32)
    comb = np.concatenate([f, s], axis=0).reshape(64, 128, F)
    in_maps = []
    for k in range(N_CORES):
        perm = [(8 * k + j) % 64 for j in range(NLOAD)]
        in_maps.append(
            {"cols_raw": np.ascontiguousarray(comb[perm].reshape(GLOB_W, F))}
        )
    return in_maps


def combine_outputs(results, first, second):
    """results: list of 8 dicts with 'out' [128, 16] and 'colsum' [1, 4864].
    first/second: the raw fp32 inputs (for the host-side gf / pos terms)."""
    R = np.zeros((64, 128))  # row-sums per absolute tile
    C = np.zeros((64, 128))  # col-sums per absolute tile
    d = np.zeros((64, 128))
    for k in range(N_CORES):
        st = np.asarray(results[k]["out"], dtype=np.float64)
        cs = np.asarray(results[k]["colsum"], dtype=np.float64).reshape(-1)
        for q in range(NROWS):
            R[8 * k + q] = st[:, S_E + q]
            d[8 * k + q] = st[:, S_D + q]
        cp = np.zeros(GLOB_W)
        cp[CS_GLO:CS_GHI] = cs
        for j in range(NLOAD):
            C[(8 * k + j) % 64] += cp[128 * j : 128 * (j + 1)]

    f = np.asarray(first, dtype=np.float64)
    s = np.asarray(second, dtype=np.float64)
    comb = np.concatenate([f, s], axis=0)
    n = comb / np.maximum(np.sqrt((comb * comb).sum(1, keepdims=True)), 1e-7)
    pos_tot = (n[:B] * n[B:]).sum()
    gf = n.sum(axis=0)

    Rf, Cf, df = R.reshape(-1), C.reshape(-1), d.reshape(-1)
    E_excl = Rf + Cf - np.exp(INV_T * df)
    lse_tot = np.log(E_excl).sum()
    raw_excl = INV_T * ((gf * gf).sum() - df.sum())
    neg = raw_excl - (TWO_B - 1) * lse_tot
    loss = -pos_tot * INV_T / B + neg / (4.0 * B * B)
    return np.asarray(loss, dtype=np.float32)


def kernel(first_transformed, second_transformed):
    nc = _get_nc()
    in_maps = make_in_maps(first_transformed, second_transformed)
    res = run_bass_kernel_spmd(nc, in_maps, core_ids=list(range(N_CORES)))
    return combine_outputs(res.results, first_transformed, second_transformed)
